# revision 3
# baseline (speedup 1.0000x reference)
"""Trainium2 Bass kernel for nn_BigramLanguageModel (8-layer dense transformer fwd).

Strategy (8 NeuronCores, no collectives):
  Launch 1 (cores 0-3, data-parallel over batch): full 8-layer trunk per batch
    row -> final-LN'd activations, feature-major bf16 [E, T] per core.
  Launch 2 (cores 0-7, vocab-sharded lm_head): each core computes a
    [4096, 4000] f32 logits slice + per-token sum(exp(logit)) partials.
  Host: embedding gather (x0 = tok_emb[idx] + pos_emb), weight prep/folding
    (LN scales folded into the following weight matrices; LN biases folded into
    per-partition bias vectors or ones-row matmul contributions), final
    log-softmax CE loss from the per-core sum-exp partials (O(B*T) work).

Trunk per-layer layout:
  - residual x: token-major f32 SBUF tiles [128t, 1024e]
  - LN (free-dim stats) -> h bf16 -> PE-transpose -> hT feature-major
  - QT/KT feature-major (lhsT=W, rhs=hT); V token-major (lhsT=hT, rhs=W) with a
    ones-column appended per head so P@[V|1] also yields the softmax row-sum
  - attention in S_T layout [k, q]: exp without max-subtraction (scores are
    O(1) here), multiplicative causal mask via affine_select on the diagonal
    block, per-head normalize after PV using reciprocal + partition_broadcast
  - attn output is feature-major => it IS the lhsT for Wo => projection comes
    out token-major; residual-add straight from PSUM. Same trick for W2.
"""
import math
import time
from contextlib import ExitStack

import numpy as np
import ml_dtypes

import jax
import concourse.bass as bass
from concourse import bacc
import concourse.mybir as mybir
import concourse.tile as tile
from concourse.masks import make_identity
from concourse import bass2jax
from concourse.bass2jax import partition_id_tensor, install_neuronx_cc_hook, _bass_exec_p
from jax.sharding import Mesh, PartitionSpec
from jax.experimental.shard_map import shard_map

bf16 = mybir.dt.bfloat16
f32 = mybir.dt.float32
AF = mybir.ActivationFunctionType
ALU = mybir.AluOpType
BF = ml_dtypes.bfloat16

B, T, E, H, HS, L, V = 4, 1024, 1024, 16, 64, 8, 32000
FF = 4 * E
NT, NE, NF = T // 128, E // 128, FF // 128
EPS = 1e-5
TT = B * T           # 4096 tokens in the head
VS = V // 8          # 4000 vocab per head core
NTB, NVT = TT // 128, VS // 500


# --------------------------------------------------------------------------
# kernel builders
# --------------------------------------------------------------------------

def build_trunk(num_devices=4):
    nc = bacc.Bacc("TRN2", target_bir_lowering=False, debug=False,
                   num_devices=num_devices)

    x0_d = nc.dram_tensor("x0", [T, E], f32, kind="ExternalInput")
    wq_d = nc.dram_tensor("wq", [L, E, E], bf16, kind="ExternalInput")
    wk_d = nc.dram_tensor("wk", [L, E, E], bf16, kind="ExternalInput")
    wv_d = nc.dram_tensor("wv", [L, E, E], bf16, kind="ExternalInput")
    wo_d = nc.dram_tensor("wo", [L, E, E], bf16, kind="ExternalInput")
    w1_d = nc.dram_tensor("w1", [L, E, FF], bf16, kind="ExternalInput")
    w2_d = nc.dram_tensor("w2", [L, FF, E], bf16, kind="ExternalInput")
    bq_d = nc.dram_tensor("bq", [L, 128, NE], f32, kind="ExternalInput")
    bk_d = nc.dram_tensor("bk", [L, 128, NE], f32, kind="ExternalInput")
    b1_d = nc.dram_tensor("b1", [L, 128, NF], f32, kind="ExternalInput")
    bo_d = nc.dram_tensor("bo", [L, 1, E], bf16, kind="ExternalInput")
    b2_d = nc.dram_tensor("b2", [L, 1, E], bf16, kind="ExternalInput")
    out_d = nc.dram_tensor("xfT", [E, T], bf16, kind="ExternalOutput")

    with ExitStack() as stack:
        tc = stack.enter_context(tile.TileContext(nc))
        pp = stack.enter_context(tc.tile_pool(name="pers", bufs=1))
        pw = stack.enter_context(tc.tile_pool(name="work", bufs=3))
        pst = stack.enter_context(tc.tile_pool(name="stats", bufs=4))
        pwt = stack.enter_context(tc.tile_pool(name="wts", bufs=2))
        pm = stack.enter_context(tc.tile_pool(name="psmm", bufs=4, space="PSUM"))
        pa = stack.enter_context(tc.tile_pool(name="psattn", bufs=2, space="PSUM"))

        identb = pp.tile([128, 128], bf16, tag="ident", name="ident")
        make_identity(nc, identb[:])
        ones_row = pp.tile([1, 128], bf16, tag="ones", name="ones")
        nc.gpsimd.memset(ones_row[:], 1.0)
        eps_t = pp.tile([128, 1], f32, tag="eps", name="eps")
        nc.gpsimd.memset(eps_t[:], EPS)

        xs = [pp.tile([128, E], f32, tag=f"x{i}", name=f"x{i}") for i in range(NT)]
        for tb in range(NT):
            nc.sync.dma_start(xs[tb][:], x0_d[tb * 128 : (tb + 1) * 128, :])

        def layer_norm_to():
            hs = []
            for tb in range(NT):
                xt = xs[tb]
                ss = pst.tile([128, 1], f32, tag="st_ss", name="st_ss")
                junk = pw.tile([128, E], bf16, tag="junk", name="junk")
                nc.scalar.activation(junk[:], xt[:], AF.Square, accum_out=ss[:])
                nsum = pst.tile([128, 1], f32, tag="st_ns", name="st_ns")
                nc.vector.tensor_reduce(
                    nsum[:], xt[:], axis=mybir.AxisListType.X, op=ALU.add, negate=True
                )
                mneg = pst.tile([128, 1], f32, tag="st_mn", name="st_mn")
                nc.vector.tensor_scalar_mul(mneg[:], nsum[:], 1.0 / E)
                m2 = pst.tile([128, 1], f32, tag="st_m2", name="st_m2")
                nc.vector.tensor_tensor(m2[:], mneg[:], mneg[:], op=ALU.mult)
                var = pst.tile([128, 1], f32, tag="st_var", name="st_var")
                nc.vector.tensor_scalar(var[:], ss[:], 1.0 / E, None, op0=ALU.mult)
                var2 = pst.tile([128, 1], f32, tag="st_var2", name="st_var2")
                nc.vector.tensor_sub(var2[:], var[:], m2[:])
                std = pst.tile([128, 1], f32, tag="st_std", name="st_std")
                nc.scalar.activation(std[:], var2[:], AF.Sqrt, bias=eps_t[:])
                rstd = pst.tile([128, 1], f32, tag="st_rstd", name="st_rstd")
                nc.vector.reciprocal(rstd[:], std[:])
                bln = pst.tile([128, 1], f32, tag="st_bln", name="st_bln")
                nc.vector.tensor_tensor(bln[:], mneg[:], rstd[:], op=ALU.mult)
                ht = pp.tile([128, E], bf16, tag=f"h{tb}", name=f"h{tb}")
                nc.scalar.activation(ht[:], xt[:], AF.Identity, bias=bln[:],
                                     scale=rstd[:])
                hs.append(ht)
            return hs

        def transpose_1024(hs):
            hT = []
            for eb in range(NE):
                t = pp.tile([128, T], bf16, tag=f"hT{eb}", name=f"hT{eb}")
                for tg in range(2):
                    ps = pm.tile([128, 512], bf16, tag="mm", name="mm")
                    for i in range(4):
                        tb = tg * 4 + i
                        nc.tensor.transpose(
                            ps[:, i * 128 : (i + 1) * 128],
                            hs[tb][:, eb * 128 : (eb + 1) * 128],
                            identb[:],
                        )
                    nc.vector.tensor_copy(t[:, tg * 512 : (tg + 1) * 512], ps[:])
                hT.append(t)
            return hT

        for l in range(L):
            # ---- LN1 + transpose ----
            hs = layer_norm_to()
            hT = transpose_1024(hs)

            # ---- QKV projections ----
            bq_t = pwt.tile([128, NE], f32, tag="bq", name="bq")
            nc.sync.dma_start(bq_t[:], bq_d[l])
            bk_t = pwt.tile([128, NE], f32, tag="bk", name="bk")
            nc.sync.dma_start(bk_t[:], bk_d[l])

            wq_t = pwt.tile([128, NE, E], bf16, tag="wqkv", name="wqkv")
            nc.sync.dma_start(wq_t[:], wq_d[l].rearrange("(eb p) d -> p eb d", p=128))
            QT = []
            for db in range(NE):
                qt = pp.tile([128, T], bf16, tag=f"qT{db}", name=f"qT{db}")
                for tcn in range(2):
                    ps = pm.tile([128, 512], f32, tag="mm", name="mm")
                    for eb in range(NE):
                        nc.tensor.matmul(
                            ps[:],
                            wq_t[:, eb, db * 128 : (db + 1) * 128],
                            hT[eb][:, tcn * 512 : (tcn + 1) * 512],
                            start=(eb == 0),
                            stop=(eb == NE - 1),
                        )
                    nc.scalar.activation(
                        qt[:, tcn * 512 : (tcn + 1) * 512], ps[:], AF.Identity,
                        bias=bq_t[:, db : db + 1],
                    )
                QT.append(qt)

            wk_t = pwt.tile([128, NE, E], bf16, tag="wqkv", name="wqkv")
            nc.sync.dma_start(wk_t[:], wk_d[l].rearrange("(eb p) d -> p eb d", p=128))
            KT = []
            for db in range(NE):
                kt = pp.tile([128, T], bf16, tag=f"kT{db}", name=f"kT{db}")
                for tcn in range(2):
                    ps = pm.tile([128, 512], f32, tag="mm", name="mm")
                    for eb in range(NE):
                        nc.tensor.matmul(
                            ps[:],
                            wk_t[:, eb, db * 128 : (db + 1) * 128],
                            hT[eb][:, tcn * 512 : (tcn + 1) * 512],
                            start=(eb == 0),
                            stop=(eb == NE - 1),
                        )
                    nc.scalar.activation(
                        kt[:, tcn * 512 : (tcn + 1) * 512], ps[:], AF.Identity,
                        bias=bk_t[:, db : db + 1],
                    )
                KT.append(kt)

            wv_t = pwt.tile([128, NE, E], bf16, tag="wqkv", name="wqkv")
            nc.sync.dma_start(wv_t[:], wv_d[l].rearrange("(eb p) d -> p eb d", p=128))
            Vs = []
            for tb in range(NT):
                vt = pp.tile([128, 16 * 65], bf16, tag=f"v{tb}", name=f"v{tb}")
                vv = vt[:].rearrange("p (h c) -> p h c", c=65)
                nc.gpsimd.memset(vv[:, :, 64:65], 1.0)
                for dc in range(2):
                    ps = pm.tile([128, 512], f32, tag="mm", name="mm")
                    for eb in range(NE):
                        nc.tensor.matmul(
                            ps[:],
                            hT[eb][:, tb * 128 : (tb + 1) * 128],
                            wv_t[:, eb, dc * 512 : (dc + 1) * 512],
                            start=(eb == 0),
                            stop=(eb == NE - 1),
                        )
                    nc.vector.tensor_copy(
                        vv[:, dc * 8 : (dc + 1) * 8, 0:64],
                        ps[:].rearrange("p (h c) -> p h c", c=64),
                    )
                Vs.append(vt)

            wo_t = pwt.tile([128, NE, E], bf16, tag="wqkv", name="wqkv")
            nc.sync.dma_start(wo_t[:], wo_d[l].rearrange("(db p) e -> p db e", p=128))
            bo_t = pwt.tile([1, E], bf16, tag="brow_o", name="brow_o")
            nc.sync.dma_start(bo_t[:], bo_d[l])

            # ---- attention ----
            attn = [None] * NE
            for h in range(H):
                db, r0 = h // 2, 64 * (h % 2)
                if h % 2 == 0:
                    attn[db] = pp.tile([128, T], bf16, tag=f"h{db}", name=f"attn{db}")
                o_ps = pa.tile([65, T], f32, tag="attn_o", name="attn_o")
                for kb in range(NT):
                    qlo = kb * 128
                    for qc in range(2):
                        lo, hi = qc * 512, qc * 512 + 512
                        if hi <= qlo:
                            continue
                        qstart = max(lo, qlo)
                        w = hi - qstart
                        sp = pm.tile([128, 512], f32, tag="mm", name="mm")
                        nc.tensor.matmul(
                            sp[:, 0:w],
                            KT[db][r0 : r0 + 64, kb * 128 : (kb + 1) * 128],
                            QT[db][r0 : r0 + 64, qstart : qstart + w],
                            start=True,
                            stop=True,
                        )
                        pt = pw.tile([128, 512], bf16, tag="pt", name="pt")
                        nc.scalar.activation(pt[:, 0:w], sp[:, 0:w], AF.Exp,
                                             scale=0.125)
                        if qstart == qlo:
                            nc.gpsimd.affine_select(
                                out=pt[:, 0:128],
                                in_=pt[:, 0:128],
                                compare_op=ALU.is_ge,
                                fill=0.0,
                                base=0,
                                pattern=[[1, 128]],
                                channel_multiplier=-1,
                            )
                        n_kb = min((hi - 1) // 128, NT - 1)
                        nc.tensor.matmul(
                            o_ps[:, qstart : qstart + w],
                            Vs[kb][:].rearrange("p (h c) -> p h c", c=65)[:, h, :],
                            pt[:, 0:w],
                            start=(kb == 0),
                            stop=(kb == n_kb),
                        )
                rec = pw.tile([1, T], f32, tag="rec", name="rec")
                nc.vector.reciprocal(rec[:], o_ps[64:65, :])
                recb = pw.tile([64, T], f32, tag="recb", name="recb")
                nc.gpsimd.partition_broadcast(recb[:], rec[:])
                nc.vector.tensor_tensor(
                    attn[db][r0 : r0 + 64, :], o_ps[0:64, :], recb[:], op=ALU.mult
                )

            # ---- output projection + residual (token-major out) ----
            for tb in range(NT):
                for ec in range(2):
                    ps = pm.tile([128, 512], f32, tag="mm", name="mm")
                    for db in range(NE):
                        nc.tensor.matmul(
                            ps[:],
                            attn[db][:, tb * 128 : (tb + 1) * 128],
                            wo_t[:, db, ec * 512 : (ec + 1) * 512],
                            start=(db == 0),
                            stop=False,
                        )
                    nc.tensor.matmul(
                        ps[:], ones_row[:], bo_t[:, ec * 512 : (ec + 1) * 512],
                        start=False, stop=True,
                    )
                    nc.vector.tensor_tensor(
                        xs[tb][:, ec * 512 : (ec + 1) * 512],
                        xs[tb][:, ec * 512 : (ec + 1) * 512],
                        ps[:],
                        op=ALU.add,
                    )

            # ---- LN2 + transpose ----
            hs2 = layer_norm_to()
            hT2 = transpose_1024(hs2)

            # ---- MLP, f-chunked so W2 accumulates into x per chunk ----
            b1_t = pwt.tile([128, NF], f32, tag="b1", name="b1")
            nc.sync.dma_start(b1_t[:], b1_d[l])
            b2_t = pwt.tile([1, E], bf16, tag="brow_b2", name="brow_b2")
            nc.sync.dma_start(b2_t[:], b2_d[l])
            for fc in range(4):
                ffs, w2s = [], []
                for j in range(8):
                    fb = fc * 8 + j
                    w1f = pwt.tile([128, NE, 128], bf16, tag="w1f", name="w1f")
                    nc.sync.dma_start(
                        w1f[:],
                        w1_d[l][:, fb * 128 : (fb + 1) * 128].rearrange(
                            "(eb p) f -> p eb f", p=128
                        ),
                    )
                    ff = pp.tile([128, T], bf16, tag=f"qT{j}", name=f"ff{j}")
                    for tcn in range(2):
                        ps = pm.tile([128, 512], f32, tag="mm", name="mm")
                        for eb in range(NE):
                            nc.tensor.matmul(
                                ps[:],
                                w1f[:, eb, :],
                                hT2[eb][:, tcn * 512 : (tcn + 1) * 512],
                                start=(eb == 0),
                                stop=(eb == NE - 1),
                            )
                        nc.scalar.activation(
                            ff[:, tcn * 512 : (tcn + 1) * 512], ps[:], AF.Relu,
                            bias=b1_t[:, fb : fb + 1],
                        )
                    ffs.append(ff)
                    w2f = pp.tile([128, E], bf16, tag=f"kT{j}", name=f"w2f{j}")
                    nc.sync.dma_start(w2f[:], w2_d[l][fb * 128 : (fb + 1) * 128, :])
                    w2s.append(w2f)
                for tb in range(NT):
                    for ec in range(2):
                        ps = pm.tile([128, 512], f32, tag="mm", name="mm")
                        for j in range(8):
                            nc.tensor.matmul(
                                ps[:],
                                ffs[j][:, tb * 128 : (tb + 1) * 128],
                                w2s[j][:, ec * 512 : (ec + 1) * 512],
                                start=(j == 0),
                                stop=(j == 7 and fc != 0),
                            )
                        if fc == 0:
                            nc.tensor.matmul(
                                ps[:], ones_row[:],
                                b2_t[:, ec * 512 : (ec + 1) * 512],
                                start=False, stop=True,
                            )
                        nc.vector.tensor_tensor(
                            xs[tb][:, ec * 512 : (ec + 1) * 512],
                            xs[tb][:, ec * 512 : (ec + 1) * 512],
                            ps[:],
                            op=ALU.add,
                        )

        # ---- final LN (pure normalize; lnf scale/bias folded into lm head) ----
        hf = layer_norm_to()
        for eb in range(NE):
            for tg in range(2):
                ps = pm.tile([128, 512], bf16, tag="mm", name="mm")
                for i in range(4):
                    tb = tg * 4 + i
                    nc.tensor.transpose(
                        ps[:, i * 128 : (i + 1) * 128],
                        hf[tb][:, eb * 128 : (eb + 1) * 128],
                        identb[:],
                    )
                ot = pw.tile([128, 512], bf16, tag="xout", name="xout")
                nc.vector.tensor_copy(ot[:], ps[:])
                nc.sync.dma_start(
                    out_d[eb * 128 : (eb + 1) * 128, tg * 512 : (tg + 1) * 512], ot[:]
                )

    nc.compile()
    return nc


def build_head(num_devices=8):
    nc = bacc.Bacc("TRN2", target_bir_lowering=False, debug=False,
                   num_devices=num_devices)

    xfT_d = nc.dram_tensor("xfT", [E, TT], bf16, kind="ExternalInput")
    wlm_d = nc.dram_tensor("wlm", [E, VS], bf16, kind="ExternalInput")
    blm_d = nc.dram_tensor("blm", [1, VS], f32, kind="ExternalInput")
    lg_d = nc.dram_tensor("logits", [TT, VS], f32, kind="ExternalOutput")
    se_d = nc.dram_tensor("se", [NTB, 128, NVT], f32, kind="ExternalOutput")

    with ExitStack() as stack:
        tc = stack.enter_context(tile.TileContext(nc))
        pp = stack.enter_context(tc.tile_pool(name="pers", bufs=1))
        pw = stack.enter_context(tc.tile_pool(name="work", bufs=3))
        pwt = stack.enter_context(tc.tile_pool(name="wts", bufs=2))
        pm = stack.enter_context(tc.tile_pool(name="psmm", bufs=4, space="PSUM"))

        xf = []
        for eb in range(NE):
            t = pp.tile([128, TT], bf16, tag=f"xf{eb}", name=f"xf{eb}")
            nc.sync.dma_start(t[:], xfT_d[eb * 128 : (eb + 1) * 128, :])
            xf.append(t)

        blm_row = pp.tile([1, VS], f32, tag="blmr", name="blmr")
        nc.sync.dma_start(blm_row[:], blm_d[:])
        blm_bc = pp.tile([128, VS], f32, tag="blmb", name="blmb")
        nc.gpsimd.partition_broadcast(blm_bc[:], blm_row[:])

        se_t = [
            pp.tile([128, NVT], f32, tag=f"se{tb}", name=f"se{tb}")
            for tb in range(NTB)
        ]

        for vt in range(NVT):
            wlm_t = pwt.tile([128, NE, 500], bf16, tag="wlm", name="wlm")
            nc.sync.dma_start(
                wlm_t[:],
                wlm_d[:, vt * 500 : (vt + 1) * 500].rearrange(
                    "(eb p) v -> p eb v", p=128
                ),
            )
            for tb in range(NTB):
                ps = pm.tile([128, 500], f32, tag="mm", name="mm")
                for eb in range(NE):
                    nc.tensor.matmul(
                        ps[:],
                        xf[eb][:, tb * 128 : (tb + 1) * 128],
                        wlm_t[:, eb, :],
                        start=(eb == 0),
                        stop=(eb == NE - 1),
                    )
                lg = pw.tile([128, 500], f32, tag="lg", name="lg")
                nc.vector.tensor_tensor(
                    lg[:], ps[:], blm_bc[:, vt * 500 : (vt + 1) * 500], op=ALU.add
                )
                nc.sync.dma_start(
                    lg_d[tb * 128 : (tb + 1) * 128, vt * 500 : (vt + 1) * 500], lg[:]
                )
                ej = pw.tile([128, 500], bf16, tag="ej", name="ej")
                nc.scalar.activation(
                    ej[:], lg[:], AF.Exp, accum_out=se_t[tb][:, vt : vt + 1]
                )
        for tb in range(NTB):
            nc.sync.dma_start(se_d[tb], se_t[tb][:])

    nc.compile()
    return nc


# --------------------------------------------------------------------------
# persistent jitted runners (compile once per process)
# --------------------------------------------------------------------------

class _Runner:
    """Wraps a Bass program as a jit-compiled shard_map callable over n cores."""

    def __init__(self, nc, n_cores):
        install_neuronx_cc_hook()
        self.nc = nc
        self.n_cores = n_cores
        part_name = nc.partition_id_tensor.name if nc.partition_id_tensor else None
        in_names, out_names, out_avals, zero_outs = [], [], [], []
        for alloc in nc.m.functions[0].allocations:
            if not isinstance(alloc, mybir.MemoryLocationSet):
                continue
            name = alloc.memorylocations[0].name
            if alloc.kind == "ExternalInput":
                if name != part_name:
                    in_names.append(name)
            elif alloc.kind == "ExternalOutput":
                out_names.append(name)
                shape = tuple(alloc.tensor_shape)
                dtype = mybir.dt.np(alloc.dtype)
                out_avals.append(jax.core.ShapedArray(shape, dtype))
                zero_outs.append(np.zeros(shape, dtype))
        self.in_names, self.out_names = in_names, out_names
        self.zero_outs = zero_outs
        n_params, n_outs = len(in_names), len(out_names)
        all_names = in_names + out_names
        if part_name is not None:
            all_names = all_names + [part_name]

        def _body(*args):
            operands = list(args)
            if part_name is not None:
                operands.append(partition_id_tensor())
            outs = _bass_exec_p.bind(
                *operands,
                out_avals=tuple(out_avals),
                in_names=tuple(all_names),
                out_names=tuple(out_names),
                lowering_input_output_aliases=(),
                sim_require_finite=True,
                sim_require_nnan=True,
                nc=nc,
            )
            return tuple(outs)

        donate = tuple(range(n_params, n_params + n_outs))
        devices = jax.devices()[:n_cores]
        assert len(devices) == n_cores
        self.devices = devices
        if n_cores == 1:
            self.fn = jax.jit(_body, donate_argnums=donate, keep_unused=True)
        else:
            mesh = Mesh(np.asarray(devices), ("core",))
            specs = (PartitionSpec("core"),) * (n_params + n_outs)
            out_specs = (PartitionSpec("core"),) * n_outs
            self.fn = jax.jit(
                shard_map(_body, mesh=mesh, in_specs=specs, out_specs=out_specs,
                          check_rep=False),
                donate_argnums=donate,
                keep_unused=True,
            )

    def concat_inputs(self, in_maps):
        if self.n_cores == 1:
            return [np.asarray(in_maps[0][n]) for n in self.in_names]
        return [
            np.concatenate([np.asarray(m[n]) for m in in_maps], axis=0)
            for n in self.in_names
        ]

    def zeros(self):
        if self.n_cores == 1:
            return list(self.zero_outs)
        return [
            np.zeros((self.n_cores * z.shape[0], *z.shape[1:]), z.dtype)
            for z in self.zero_outs
        ]

    def __call__(self, in_maps):
        arrs = self.fn(*self.concat_inputs(in_maps), *self.zeros())
        out = []
        for c in range(self.n_cores):
            d = {}
            for i, n in enumerate(self.out_names):
                a = np.asarray(arrs[i])
                if self.n_cores > 1:
                    a = a.reshape(self.n_cores, -1, *a.shape[1:])[c]
                d[n] = a
            out.append(d)
        return out


_CACHE = {}


def _get_runner(kind):
    if kind not in _CACHE:
        if kind == "trunk":
            _CACHE[kind] = _Runner(build_trunk(num_devices=4), 4)
        else:
            _CACHE[kind] = _Runner(build_head(num_devices=8), 8)
    return _CACHE[kind]


# --------------------------------------------------------------------------
# host-side prep
# --------------------------------------------------------------------------

def _prep_trunk_weights(Wq, Wk, Wv, Wo, bo, ln1s, ln1b, ln2s, ln2b, W1, b1, W2, b2):
    wq = (Wq * ln1s[:, :, None]).astype(BF)
    wk = (Wk * ln1s[:, :, None]).astype(BF)
    wv = (Wv * ln1s[:, :, None]).astype(BF)
    bq = np.einsum("le,led->ld", ln1b, Wq).astype(np.float32)
    bk = np.einsum("le,led->ld", ln1b, Wk).astype(np.float32)
    bv = np.einsum("le,led->ld", ln1b, Wv).astype(np.float32)
    bo_fold = (bo + np.einsum("ld,lde->le", bv, Wo)).astype(np.float32)
    w1 = (W1 * ln2s[:, :, None]).astype(BF)
    b1_fold = (b1 + np.einsum("le,lef->lf", ln2b, W1)).astype(np.float32)

    def to_p8(b):
        return np.ascontiguousarray(b.reshape(L, -1, 128).transpose(0, 2, 1))

    return {
        "wq": wq, "wk": wk, "wv": wv, "wo": Wo.astype(BF),
        "w1": w1, "w2": W2.astype(BF),
        "bq": to_p8(bq), "bk": to_p8(bk), "b1": to_p8(b1_fold),
        "bo": bo_fold.reshape(L, 1, E).astype(BF),
        "b2": b2.reshape(L, 1, E).astype(BF),
    }


def kernel(idx, targets, tok_emb, pos_emb, Wq, Wk, Wv, Wo, bo,
           ln1s, ln1b, ln2s, ln2b, W1, b1, W2, b2, lnfs, lnfb, Wlm, blm):
    idx = np.asarray(idx)
    targets = np.asarray(targets)
    to_np = lambda a: np.asarray(a, dtype=np.float32)
    tok_emb, pos_emb = to_np(tok_emb), to_np(pos_emb)
    Wq, Wk, Wv, Wo, bo = map(to_np, (Wq, Wk, Wv, Wo, bo))
    ln1s, ln1b, ln2s, ln2b = map(to_np, (ln1s, ln1b, ln2s, ln2b))
    W1, b1, W2, b2 = map(to_np, (W1, b1, W2, b2))
    lnfs, lnfb, Wlm, blm = map(to_np, (lnfs, lnfb, Wlm, blm))

    # host embedding gather
    x0 = tok_emb[idx] + pos_emb[None, :T]          # [B, T, E] f32

    shared = _prep_trunk_weights(Wq, Wk, Wv, Wo, bo, ln1s, ln1b, ln2s, ln2b,
                                 W1, b1, W2, b2)
    trunk = _get_runner("trunk")
    in_maps = [dict(shared, x0=np.ascontiguousarray(x0[b])) for b in range(B)]
    t0 = time.time()
    tr = trunk(in_maps)
    kernel.trunk_wall = time.time() - t0

    xfT_all = np.concatenate([r["xfT"].astype(BF) for r in tr], axis=1)  # [E, TT]

    wlm_full = (Wlm * lnfs[:, None]).astype(BF)
    blm_full = (blm + lnfb @ Wlm).astype(np.float32)
    head = _get_runner("head")
    hmaps = [
        {
            "xfT": xfT_all,
            "wlm": np.ascontiguousarray(wlm_full[:, c * VS : (c + 1) * VS]),
            "blm": blm_full[c * VS : (c + 1) * VS].reshape(1, VS),
        }
        for c in range(8)
    ]
    t0 = time.time()
    hr = head(hmaps)
    kernel.head_wall = time.time() - t0

    logits = np.concatenate([r["logits"] for r in hr], axis=1)  # [TT, V] f32
    sumexp = np.sum([r["se"].sum(-1).reshape(TT) for r in hr], axis=0)
    lse = np.log(sumexp)
    tflat = targets.reshape(TT).astype(np.int64)
    nll = lse - logits[np.arange(TT), tflat]
    loss = np.float32(nll.mean())
    return logits.reshape(B, T, V), loss


# revision 6
# speedup vs baseline: 4.6788x; 4.6788x over previous
"""Trainium2 Bass kernel for nn_BigramLanguageModel (8-layer dense transformer fwd).

Strategy (8 NeuronCores, no collectives):
  Launch 1 (cores 0-3, data-parallel over batch): full 8-layer trunk per batch
    row -> final-LN'd activations, feature-major bf16 [E, T] per core.
  Launch 2 (cores 0-7, vocab-sharded lm_head): each core computes a
    [4096, 4000] f32 logits slice + per-token sum(exp(logit)) partials.
  Host: embedding gather (x0 = tok_emb[idx] + pos_emb), weight prep/folding
    (LN scales folded into the following weight matrices; LN biases folded into
    per-partition bias vectors or ones-row matmul contributions), final
    log-softmax CE loss from the per-core sum-exp partials (O(B*T) work).

Trunk per-layer layout:
  - residual x: token-major f32 SBUF tiles [128t, 1024e]
  - LN (free-dim stats) -> h bf16 -> PE-transpose -> hT feature-major
  - QT/KT feature-major (lhsT=W, rhs=hT); V token-major (lhsT=hT, rhs=W) with a
    ones-column appended per head so P@[V|1] also yields the softmax row-sum
  - attention in S_T layout [k, q]: exp without max-subtraction (scores are
    O(1) here), multiplicative causal mask via affine_select on the diagonal
    block, per-head normalize after PV using reciprocal + partition_broadcast
  - attn output is feature-major => it IS the lhsT for Wo => projection comes
    out token-major; residual-add straight from PSUM. Same trick for W2.
"""
import math
import time
from contextlib import ExitStack

import numpy as np
import ml_dtypes

import jax
import concourse.bass as bass
from concourse import bacc
import concourse.mybir as mybir
import concourse.tile as tile
from concourse.masks import make_identity
from concourse import bass2jax
from concourse.bass2jax import partition_id_tensor, install_neuronx_cc_hook, _bass_exec_p
from jax.sharding import Mesh, PartitionSpec
from jax.experimental.shard_map import shard_map

bf16 = mybir.dt.bfloat16
f32 = mybir.dt.float32
AF = mybir.ActivationFunctionType
ALU = mybir.AluOpType
BF = ml_dtypes.bfloat16

B, T, E, H, HS, L, V = 4, 1024, 1024, 16, 64, 8, 32000
FF = 4 * E
NT, NE, NF = T // 128, E // 128, FF // 128
EPS = 1e-5
TT = B * T           # 4096 tokens in the head
VS = V // 8          # 4000 vocab per head core
NTB, NVT = TT // 128, VS // 500


# --------------------------------------------------------------------------
# kernel builders
# --------------------------------------------------------------------------

def build_trunk(num_devices=4):
    nc = bacc.Bacc("TRN2", target_bir_lowering=False, debug=False,
                   num_devices=num_devices)

    x0_d = nc.dram_tensor("x0", [T, E], f32, kind="ExternalInput")
    wq_d = nc.dram_tensor("wq", [L, E, E], bf16, kind="ExternalInput")
    wk_d = nc.dram_tensor("wk", [L, E, E], bf16, kind="ExternalInput")
    wv_d = nc.dram_tensor("wv", [L, E, E], bf16, kind="ExternalInput")
    wo_d = nc.dram_tensor("wo", [L, E, E], bf16, kind="ExternalInput")
    w1_d = nc.dram_tensor("w1", [L, E, FF], bf16, kind="ExternalInput")
    w2_d = nc.dram_tensor("w2", [L, FF, E], bf16, kind="ExternalInput")
    bq_d = nc.dram_tensor("bq", [L, 128, NE], f32, kind="ExternalInput")
    bk_d = nc.dram_tensor("bk", [L, 128, NE], f32, kind="ExternalInput")
    b1_d = nc.dram_tensor("b1", [L, 128, NF], f32, kind="ExternalInput")
    bo_d = nc.dram_tensor("bo", [L, 1, E], bf16, kind="ExternalInput")
    b2_d = nc.dram_tensor("b2", [L, 1, E], bf16, kind="ExternalInput")
    out_d = nc.dram_tensor("xfT", [E, T], bf16, kind="ExternalOutput")

    with ExitStack() as stack:
        tc = stack.enter_context(tile.TileContext(nc))
        pp = stack.enter_context(tc.tile_pool(name="pers", bufs=1))
        pw = stack.enter_context(tc.tile_pool(name="work", bufs=3))
        pst = stack.enter_context(tc.tile_pool(name="stats", bufs=4))
        pwt = stack.enter_context(tc.tile_pool(name="wts", bufs=2))
        pm = stack.enter_context(tc.tile_pool(name="psmm", bufs=4, space="PSUM"))
        pa = stack.enter_context(tc.tile_pool(name="psattn", bufs=2, space="PSUM"))

        identb = pp.tile([128, 128], bf16, tag="ident", name="ident")
        make_identity(nc, identb[:])
        ones_row = pp.tile([1, 128], bf16, tag="ones", name="ones")
        nc.gpsimd.memset(ones_row[:], 1.0)
        eps_t = pp.tile([128, 1], f32, tag="eps", name="eps")
        nc.gpsimd.memset(eps_t[:], EPS)

        xs = [pp.tile([128, E], f32, tag=f"x{i}", name=f"x{i}") for i in range(NT)]
        for tb in range(NT):
            nc.sync.dma_start(xs[tb][:], x0_d[tb * 128 : (tb + 1) * 128, :])

        def layer_norm_to():
            hs = []
            for tb in range(NT):
                xt = xs[tb]
                ss = pst.tile([128, 1], f32, tag="st_ss", name="st_ss")
                junk = pw.tile([128, E], bf16, tag="junk", name="junk")
                nc.scalar.activation(junk[:], xt[:], AF.Square, accum_out=ss[:])
                nsum = pst.tile([128, 1], f32, tag="st_ns", name="st_ns")
                nc.vector.tensor_reduce(
                    nsum[:], xt[:], axis=mybir.AxisListType.X, op=ALU.add, negate=True
                )
                mneg = pst.tile([128, 1], f32, tag="st_mn", name="st_mn")
                nc.vector.tensor_scalar_mul(mneg[:], nsum[:], 1.0 / E)
                m2 = pst.tile([128, 1], f32, tag="st_m2", name="st_m2")
                nc.vector.tensor_tensor(m2[:], mneg[:], mneg[:], op=ALU.mult)
                var = pst.tile([128, 1], f32, tag="st_var", name="st_var")
                nc.vector.tensor_scalar(var[:], ss[:], 1.0 / E, None, op0=ALU.mult)
                var2 = pst.tile([128, 1], f32, tag="st_var2", name="st_var2")
                nc.vector.tensor_sub(var2[:], var[:], m2[:])
                std = pst.tile([128, 1], f32, tag="st_std", name="st_std")
                nc.scalar.activation(std[:], var2[:], AF.Sqrt, bias=eps_t[:])
                rstd = pst.tile([128, 1], f32, tag="st_rstd", name="st_rstd")
                nc.vector.reciprocal(rstd[:], std[:])
                bln = pst.tile([128, 1], f32, tag="st_bln", name="st_bln")
                nc.vector.tensor_tensor(bln[:], mneg[:], rstd[:], op=ALU.mult)
                ht = pp.tile([128, E], bf16, tag=f"h{tb}", name=f"h{tb}")
                nc.scalar.activation(ht[:], xt[:], AF.Identity, bias=bln[:],
                                     scale=rstd[:])
                hs.append(ht)
            return hs

        def transpose_1024(hs):
            hT = []
            for eb in range(NE):
                t = pp.tile([128, T], bf16, tag=f"hT{eb}", name=f"hT{eb}")
                for tg in range(2):
                    ps = pm.tile([128, 512], bf16, tag="mm", name="mm")
                    for i in range(4):
                        tb = tg * 4 + i
                        nc.tensor.transpose(
                            ps[:, i * 128 : (i + 1) * 128],
                            hs[tb][:, eb * 128 : (eb + 1) * 128],
                            identb[:],
                        )
                    nc.vector.tensor_copy(t[:, tg * 512 : (tg + 1) * 512], ps[:])
                hT.append(t)
            return hT

        for l in range(L):
            # ---- LN1 + transpose ----
            hs = layer_norm_to()
            hT = transpose_1024(hs)

            # ---- QKV projections ----
            bq_t = pwt.tile([128, NE], f32, tag="bq", name="bq")
            nc.sync.dma_start(bq_t[:], bq_d[l])
            bk_t = pwt.tile([128, NE], f32, tag="bk", name="bk")
            nc.sync.dma_start(bk_t[:], bk_d[l])

            wq_t = pwt.tile([128, NE, E], bf16, tag="wqkv", name="wqkv")
            nc.sync.dma_start(wq_t[:], wq_d[l].rearrange("(eb p) d -> p eb d", p=128))
            QT = []
            for db in range(NE):
                qt = pp.tile([128, T], bf16, tag=f"qT{db}", name=f"qT{db}")
                for tcn in range(2):
                    ps = pm.tile([128, 512], f32, tag="mm", name="mm")
                    for eb in range(NE):
                        nc.tensor.matmul(
                            ps[:],
                            wq_t[:, eb, db * 128 : (db + 1) * 128],
                            hT[eb][:, tcn * 512 : (tcn + 1) * 512],
                            start=(eb == 0),
                            stop=(eb == NE - 1),
                        )
                    nc.scalar.activation(
                        qt[:, tcn * 512 : (tcn + 1) * 512], ps[:], AF.Identity,
                        bias=bq_t[:, db : db + 1],
                    )
                QT.append(qt)

            wk_t = pwt.tile([128, NE, E], bf16, tag="wqkv", name="wqkv")
            nc.sync.dma_start(wk_t[:], wk_d[l].rearrange("(eb p) d -> p eb d", p=128))
            KT = []
            for db in range(NE):
                kt = pp.tile([128, T], bf16, tag=f"kT{db}", name=f"kT{db}")
                for tcn in range(2):
                    ps = pm.tile([128, 512], f32, tag="mm", name="mm")
                    for eb in range(NE):
                        nc.tensor.matmul(
                            ps[:],
                            wk_t[:, eb, db * 128 : (db + 1) * 128],
                            hT[eb][:, tcn * 512 : (tcn + 1) * 512],
                            start=(eb == 0),
                            stop=(eb == NE - 1),
                        )
                    nc.scalar.activation(
                        kt[:, tcn * 512 : (tcn + 1) * 512], ps[:], AF.Identity,
                        bias=bk_t[:, db : db + 1],
                    )
                KT.append(kt)

            wv_t = pwt.tile([128, NE, E], bf16, tag="wqkv", name="wqkv")
            nc.sync.dma_start(wv_t[:], wv_d[l].rearrange("(eb p) d -> p eb d", p=128))
            Vs = []
            for tb in range(NT):
                vt = pp.tile([128, 16 * 65], bf16, tag=f"v{tb}", name=f"v{tb}")
                vv = vt[:].rearrange("p (h c) -> p h c", c=65)
                nc.gpsimd.memset(vv[:, :, 64:65], 1.0)
                for dc in range(2):
                    ps = pm.tile([128, 512], f32, tag="mm", name="mm")
                    for eb in range(NE):
                        nc.tensor.matmul(
                            ps[:],
                            hT[eb][:, tb * 128 : (tb + 1) * 128],
                            wv_t[:, eb, dc * 512 : (dc + 1) * 512],
                            start=(eb == 0),
                            stop=(eb == NE - 1),
                        )
                    nc.vector.tensor_copy(
                        vv[:, dc * 8 : (dc + 1) * 8, 0:64],
                        ps[:].rearrange("p (h c) -> p h c", c=64),
                    )
                Vs.append(vt)

            wo_t = pwt.tile([128, NE, E], bf16, tag="wqkv", name="wqkv")
            nc.sync.dma_start(wo_t[:], wo_d[l].rearrange("(db p) e -> p db e", p=128))
            bo_t = pwt.tile([1, E], bf16, tag="brow_o", name="brow_o")
            nc.sync.dma_start(bo_t[:], bo_d[l])

            # ---- attention ----
            attn = [None] * NE
            for h in range(H):
                db, r0 = h // 2, 64 * (h % 2)
                if h % 2 == 0:
                    attn[db] = pp.tile([128, T], bf16, tag=f"h{db}", name=f"attn{db}")
                o_ps = pa.tile([65, T], f32, tag="attn_o", name="attn_o")
                for kb in range(NT):
                    qlo = kb * 128
                    for qc in range(2):
                        lo, hi = qc * 512, qc * 512 + 512
                        if hi <= qlo:
                            continue
                        qstart = max(lo, qlo)
                        w = hi - qstart
                        sp = pm.tile([128, 512], f32, tag="mm", name="mm")
                        nc.tensor.matmul(
                            sp[:, 0:w],
                            KT[db][r0 : r0 + 64, kb * 128 : (kb + 1) * 128],
                            QT[db][r0 : r0 + 64, qstart : qstart + w],
                            start=True,
                            stop=True,
                        )
                        pt = pw.tile([128, 512], bf16, tag="pt", name="pt")
                        nc.scalar.activation(pt[:, 0:w], sp[:, 0:w], AF.Exp,
                                             scale=0.125)
                        if qstart == qlo:
                            nc.gpsimd.affine_select(
                                out=pt[:, 0:128],
                                in_=pt[:, 0:128],
                                compare_op=ALU.is_ge,
                                fill=0.0,
                                base=0,
                                pattern=[[1, 128]],
                                channel_multiplier=-1,
                            )
                        n_kb = min((hi - 1) // 128, NT - 1)
                        nc.tensor.matmul(
                            o_ps[:, qstart : qstart + w],
                            Vs[kb][:].rearrange("p (h c) -> p h c", c=65)[:, h, :],
                            pt[:, 0:w],
                            start=(kb == 0),
                            stop=(kb == n_kb),
                        )
                rec = pw.tile([1, T], f32, tag="rec", name="rec")
                nc.vector.reciprocal(rec[:], o_ps[64:65, :])
                recb = pw.tile([64, T], f32, tag="recb", name="recb")
                nc.gpsimd.partition_broadcast(recb[:], rec[:])
                nc.vector.tensor_tensor(
                    attn[db][r0 : r0 + 64, :], o_ps[0:64, :], recb[:], op=ALU.mult
                )

            # ---- output projection + residual (token-major out) ----
            for tb in range(NT):
                for ec in range(2):
                    ps = pm.tile([128, 512], f32, tag="mm", name="mm")
                    for db in range(NE):
                        nc.tensor.matmul(
                            ps[:],
                            attn[db][:, tb * 128 : (tb + 1) * 128],
                            wo_t[:, db, ec * 512 : (ec + 1) * 512],
                            start=(db == 0),
                            stop=False,
                        )
                    nc.tensor.matmul(
                        ps[:], ones_row[:], bo_t[:, ec * 512 : (ec + 1) * 512],
                        start=False, stop=True,
                    )
                    nc.vector.tensor_tensor(
                        xs[tb][:, ec * 512 : (ec + 1) * 512],
                        xs[tb][:, ec * 512 : (ec + 1) * 512],
                        ps[:],
                        op=ALU.add,
                    )

            # ---- LN2 + transpose ----
            hs2 = layer_norm_to()
            hT2 = transpose_1024(hs2)

            # ---- MLP, f-chunked so W2 accumulates into x per chunk ----
            b1_t = pwt.tile([128, NF], f32, tag="b1", name="b1")
            nc.sync.dma_start(b1_t[:], b1_d[l])
            b2_t = pwt.tile([1, E], bf16, tag="brow_b2", name="brow_b2")
            nc.sync.dma_start(b2_t[:], b2_d[l])
            for fc in range(4):
                ffs, w2s = [], []
                for j in range(8):
                    fb = fc * 8 + j
                    w1f = pwt.tile([128, NE, 128], bf16, tag="w1f", name="w1f")
                    nc.sync.dma_start(
                        w1f[:],
                        w1_d[l][:, fb * 128 : (fb + 1) * 128].rearrange(
                            "(eb p) f -> p eb f", p=128
                        ),
                    )
                    ff = pp.tile([128, T], bf16, tag=f"qT{j}", name=f"ff{j}")
                    for tcn in range(2):
                        ps = pm.tile([128, 512], f32, tag="mm", name="mm")
                        for eb in range(NE):
                            nc.tensor.matmul(
                                ps[:],
                                w1f[:, eb, :],
                                hT2[eb][:, tcn * 512 : (tcn + 1) * 512],
                                start=(eb == 0),
                                stop=(eb == NE - 1),
                            )
                        nc.scalar.activation(
                            ff[:, tcn * 512 : (tcn + 1) * 512], ps[:], AF.Relu,
                            bias=b1_t[:, fb : fb + 1],
                        )
                    ffs.append(ff)
                    w2f = pp.tile([128, E], bf16, tag=f"kT{j}", name=f"w2f{j}")
                    nc.sync.dma_start(w2f[:], w2_d[l][fb * 128 : (fb + 1) * 128, :])
                    w2s.append(w2f)
                for tb in range(NT):
                    for ec in range(2):
                        ps = pm.tile([128, 512], f32, tag="mm", name="mm")
                        for j in range(8):
                            nc.tensor.matmul(
                                ps[:],
                                ffs[j][:, tb * 128 : (tb + 1) * 128],
                                w2s[j][:, ec * 512 : (ec + 1) * 512],
                                start=(j == 0),
                                stop=(j == 7 and fc != 0),
                            )
                        if fc == 0:
                            nc.tensor.matmul(
                                ps[:], ones_row[:],
                                b2_t[:, ec * 512 : (ec + 1) * 512],
                                start=False, stop=True,
                            )
                        nc.vector.tensor_tensor(
                            xs[tb][:, ec * 512 : (ec + 1) * 512],
                            xs[tb][:, ec * 512 : (ec + 1) * 512],
                            ps[:],
                            op=ALU.add,
                        )

        # ---- final LN (pure normalize; lnf scale/bias folded into lm head) ----
        hf = layer_norm_to()
        for eb in range(NE):
            for tg in range(2):
                ps = pm.tile([128, 512], bf16, tag="mm", name="mm")
                for i in range(4):
                    tb = tg * 4 + i
                    nc.tensor.transpose(
                        ps[:, i * 128 : (i + 1) * 128],
                        hf[tb][:, eb * 128 : (eb + 1) * 128],
                        identb[:],
                    )
                ot = pw.tile([128, 512], bf16, tag="xout", name="xout")
                nc.vector.tensor_copy(ot[:], ps[:])
                nc.sync.dma_start(
                    out_d[eb * 128 : (eb + 1) * 128, tg * 512 : (tg + 1) * 512], ot[:]
                )

    nc.compile()
    return nc


def build_head(num_devices=8):
    nc = bacc.Bacc("TRN2", target_bir_lowering=False, debug=False,
                   num_devices=num_devices)

    xfT_d = nc.dram_tensor("xfT", [E, TT], bf16, kind="ExternalInput")
    wlm_d = nc.dram_tensor("wlm", [E, VS], bf16, kind="ExternalInput")
    blm_d = nc.dram_tensor("blm", [1, VS], f32, kind="ExternalInput")
    lg_d = nc.dram_tensor("logits", [TT, VS], f32, kind="ExternalOutput")
    se_d = nc.dram_tensor("se", [NTB, 128, NVT], f32, kind="ExternalOutput")

    with ExitStack() as stack:
        tc = stack.enter_context(tile.TileContext(nc))
        pp = stack.enter_context(tc.tile_pool(name="pers", bufs=1))
        pw = stack.enter_context(tc.tile_pool(name="work", bufs=3))
        pwt = stack.enter_context(tc.tile_pool(name="wts", bufs=2))
        pm = stack.enter_context(tc.tile_pool(name="psmm", bufs=4, space="PSUM"))

        xf = []
        for eb in range(NE):
            t = pp.tile([128, TT], bf16, tag=f"xf{eb}", name=f"xf{eb}")
            nc.sync.dma_start(t[:], xfT_d[eb * 128 : (eb + 1) * 128, :])
            xf.append(t)

        blm_row = pp.tile([1, VS], f32, tag="blmr", name="blmr")
        nc.sync.dma_start(blm_row[:], blm_d[:])
        blm_bc = pp.tile([128, VS], f32, tag="blmb", name="blmb")
        nc.gpsimd.partition_broadcast(blm_bc[:], blm_row[:])

        se_t = [
            pp.tile([128, NVT], f32, tag=f"se{tb}", name=f"se{tb}")
            for tb in range(NTB)
        ]

        for vt in range(NVT):
            wlm_t = pwt.tile([128, NE, 500], bf16, tag="wlm", name="wlm")
            nc.sync.dma_start(
                wlm_t[:],
                wlm_d[:, vt * 500 : (vt + 1) * 500].rearrange(
                    "(eb p) v -> p eb v", p=128
                ),
            )
            for tb in range(NTB):
                ps = pm.tile([128, 500], f32, tag="mm", name="mm")
                for eb in range(NE):
                    nc.tensor.matmul(
                        ps[:],
                        xf[eb][:, tb * 128 : (tb + 1) * 128],
                        wlm_t[:, eb, :],
                        start=(eb == 0),
                        stop=(eb == NE - 1),
                    )
                lg = pw.tile([128, 500], f32, tag="lg", name="lg")
                nc.vector.tensor_tensor(
                    lg[:], ps[:], blm_bc[:, vt * 500 : (vt + 1) * 500], op=ALU.add
                )
                nc.sync.dma_start(
                    lg_d[tb * 128 : (tb + 1) * 128, vt * 500 : (vt + 1) * 500], lg[:]
                )
                ej = pw.tile([128, 500], bf16, tag="ej", name="ej")
                nc.scalar.activation(
                    ej[:], lg[:], AF.Exp, accum_out=se_t[tb][:, vt : vt + 1]
                )
        for tb in range(NTB):
            nc.sync.dma_start(se_d[tb], se_t[tb][:])

    nc.compile()
    return nc


# --------------------------------------------------------------------------
# persistent jitted runners (compile once per process)
# --------------------------------------------------------------------------

class _Runner:
    """Wraps a Bass program as a jit-compiled shard_map callable over n cores."""

    def __init__(self, nc, n_cores):
        install_neuronx_cc_hook()
        self.nc = nc
        self.n_cores = n_cores
        part_name = nc.partition_id_tensor.name if nc.partition_id_tensor else None
        in_names, out_names, out_avals, zero_outs = [], [], [], []
        for alloc in nc.m.functions[0].allocations:
            if not isinstance(alloc, mybir.MemoryLocationSet):
                continue
            name = alloc.memorylocations[0].name
            if alloc.kind == "ExternalInput":
                if name != part_name:
                    in_names.append(name)
            elif alloc.kind == "ExternalOutput":
                out_names.append(name)
                shape = tuple(alloc.tensor_shape)
                dtype = mybir.dt.np(alloc.dtype)
                out_avals.append(jax.core.ShapedArray(shape, dtype))
                zero_outs.append(np.zeros(shape, dtype))
        self.in_names, self.out_names = in_names, out_names
        self.zero_outs = zero_outs
        n_params, n_outs = len(in_names), len(out_names)
        all_names = in_names + out_names
        if part_name is not None:
            all_names = all_names + [part_name]

        def _body(*args):
            operands = list(args)
            if part_name is not None:
                operands.append(partition_id_tensor())
            outs = _bass_exec_p.bind(
                *operands,
                out_avals=tuple(out_avals),
                in_names=tuple(all_names),
                out_names=tuple(out_names),
                lowering_input_output_aliases=(),
                sim_require_finite=True,
                sim_require_nnan=True,
                nc=nc,
            )
            return tuple(outs)

        donate = tuple(range(n_params, n_params + n_outs))
        devices = jax.devices()[:n_cores]
        assert len(devices) == n_cores
        self.devices = devices
        self.mesh = None
        if n_cores == 1:
            self.fn = jax.jit(_body, donate_argnums=donate, keep_unused=True)
        else:
            mesh = Mesh(np.asarray(devices), ("core",))
            self.mesh = mesh
            specs = (PartitionSpec("core"),) * (n_params + n_outs)
            out_specs = (PartitionSpec("core"),) * n_outs
            self.fn = jax.jit(
                shard_map(_body, mesh=mesh, in_specs=specs, out_specs=out_specs,
                          check_rep=False),
                donate_argnums=donate,
                keep_unused=True,
            )

    def device_args(self, in_maps):
        """device_put inputs + fresh zero outputs; returns list of jax arrays."""
        arrs = self.concat_inputs(in_maps) + self.zeros()
        if self.mesh is None:
            return [jax.device_put(a, self.devices[0]) for a in arrs]
        sh = jax.sharding.NamedSharding(self.mesh, PartitionSpec("core"))
        return [jax.device_put(a, sh) for a in arrs]

    def bench(self, in_maps, n=5):
        """Time the jitted call with device-resident args. Returns min seconds."""
        times = []
        for _ in range(n):
            args = self.device_args(in_maps)
            t0 = time.time()
            out = self.fn(*args)
            jax.block_until_ready(out)
            times.append(time.time() - t0)
            del out
        return min(times)

    def concat_inputs(self, in_maps):
        if self.n_cores == 1:
            return [np.asarray(in_maps[0][n]) for n in self.in_names]
        return [
            np.concatenate([np.asarray(m[n]) for m in in_maps], axis=0)
            for n in self.in_names
        ]

    def zeros(self):
        if self.n_cores == 1:
            return list(self.zero_outs)
        return [
            np.zeros((self.n_cores * z.shape[0], *z.shape[1:]), z.dtype)
            for z in self.zero_outs
        ]

    def __call__(self, in_maps):
        arrs = self.fn(*self.concat_inputs(in_maps), *self.zeros())
        out = []
        for c in range(self.n_cores):
            d = {}
            for i, n in enumerate(self.out_names):
                a = np.asarray(arrs[i])
                if self.n_cores > 1:
                    a = a.reshape(self.n_cores, -1, *a.shape[1:])[c]
                d[n] = a
            out.append(d)
        return out


_CACHE = {}


def _get_runner(kind):
    if kind not in _CACHE:
        if kind == "trunk":
            _CACHE[kind] = _Runner(build_trunk(num_devices=4), 4)
        else:
            _CACHE[kind] = _Runner(build_head(num_devices=8), 8)
    return _CACHE[kind]


# --------------------------------------------------------------------------
# host-side prep
# --------------------------------------------------------------------------

def _prep_trunk_weights(Wq, Wk, Wv, Wo, bo, ln1s, ln1b, ln2s, ln2b, W1, b1, W2, b2):
    wq = (Wq * ln1s[:, :, None]).astype(BF)
    wk = (Wk * ln1s[:, :, None]).astype(BF)
    wv = (Wv * ln1s[:, :, None]).astype(BF)
    bq = np.einsum("le,led->ld", ln1b, Wq).astype(np.float32)
    bk = np.einsum("le,led->ld", ln1b, Wk).astype(np.float32)
    bv = np.einsum("le,led->ld", ln1b, Wv).astype(np.float32)
    bo_fold = (bo + np.einsum("ld,lde->le", bv, Wo)).astype(np.float32)
    w1 = (W1 * ln2s[:, :, None]).astype(BF)
    b1_fold = (b1 + np.einsum("le,lef->lf", ln2b, W1)).astype(np.float32)

    def to_p8(b):
        return np.ascontiguousarray(b.reshape(L, -1, 128).transpose(0, 2, 1))

    return {
        "wq": wq, "wk": wk, "wv": wv, "wo": Wo.astype(BF),
        "w1": w1, "w2": W2.astype(BF),
        "bq": to_p8(bq), "bk": to_p8(bk), "b1": to_p8(b1_fold),
        "bo": bo_fold.reshape(L, 1, E).astype(BF),
        "b2": b2.reshape(L, 1, E).astype(BF),
    }


def kernel(idx, targets, tok_emb, pos_emb, Wq, Wk, Wv, Wo, bo,
           ln1s, ln1b, ln2s, ln2b, W1, b1, W2, b2, lnfs, lnfb, Wlm, blm):
    idx = np.asarray(idx)
    targets = np.asarray(targets)
    to_np = lambda a: np.asarray(a, dtype=np.float32)
    tok_emb, pos_emb = to_np(tok_emb), to_np(pos_emb)
    Wq, Wk, Wv, Wo, bo = map(to_np, (Wq, Wk, Wv, Wo, bo))
    ln1s, ln1b, ln2s, ln2b = map(to_np, (ln1s, ln1b, ln2s, ln2b))
    W1, b1, W2, b2 = map(to_np, (W1, b1, W2, b2))
    lnfs, lnfb, Wlm, blm = map(to_np, (lnfs, lnfb, Wlm, blm))

    # host embedding gather
    x0 = tok_emb[idx] + pos_emb[None, :T]          # [B, T, E] f32

    shared = _prep_trunk_weights(Wq, Wk, Wv, Wo, bo, ln1s, ln1b, ln2s, ln2b,
                                 W1, b1, W2, b2)
    trunk = _get_runner("trunk")
    in_maps = [dict(shared, x0=np.ascontiguousarray(x0[b])) for b in range(B)]
    kernel.last_trunk_maps = in_maps
    t0 = time.time()
    tr = trunk(in_maps)
    kernel.trunk_wall = time.time() - t0

    xfT_all = np.concatenate([r["xfT"].astype(BF) for r in tr], axis=1)  # [E, TT]

    wlm_full = (Wlm * lnfs[:, None]).astype(BF)
    blm_full = (blm + lnfb @ Wlm).astype(np.float32)
    head = _get_runner("head")
    hmaps = [
        {
            "xfT": xfT_all,
            "wlm": np.ascontiguousarray(wlm_full[:, c * VS : (c + 1) * VS]),
            "blm": blm_full[c * VS : (c + 1) * VS].reshape(1, VS),
        }
        for c in range(8)
    ]
    kernel.last_head_maps = hmaps
    t0 = time.time()
    hr = head(hmaps)
    kernel.head_wall = time.time() - t0

    logits = np.concatenate([r["logits"] for r in hr], axis=1)  # [TT, V] f32
    sumexp = np.sum([r["se"].sum(-1).reshape(TT) for r in hr], axis=0)
    lse = np.log(sumexp)
    tflat = targets.reshape(TT).astype(np.int64)
    nll = lse - logits[np.arange(TT), tflat]
    loss = np.float32(nll.mean())
    return logits.reshape(B, T, V), loss
